# revision 1
# baseline (speedup 1.0000x reference)
"""Criss-cross (CCNet) attention kernel for Trainium2, 8 NeuronCores.

Sharding: core c in 0..7 -> batch b = c//2, value-channel half h = c%2.
Each core computes, for its (b, h): the full joint row+column softmax
attention with 256 of the 512 value/output channels.

Device-side math (per core), H = W = 128, Cqk = 64, Cv = 256:
  - q,k = Wq x, Wk x   (fp32r matmuls, kept in SBUF as stacked [128, H*W])
  - row pass, per row y:  E[i,x] = sum_c k[c,y,i] q[c,y,x]; P = exp(E)
      O[x, 0:256] = P^T V_y, O[x,256:258] = row-sums of P (ones columns)
      orow[y*128+x, c] = O[x, c] / O[x, 256]   (fp16), rs_row = 1/O[:,256]
  - col pass, per col x:  E[j,y] = sum_c k[c,j,x] q[c,y,x]; P = exp(E) with
      diagonal j==y masked to 0; same aggregation against V columns.
  - v staged to DRAM (fp32r) between the passes (row-major write, column
    gather read).
Host combines the two passes:  out = (o_r*s_r + o_c*s_c) / (s_r + s_c)
using the returned reciprocal sums rs = 1/s of both passes.

Performance notes (per core, HW-profiled):
  - all matmuls fp32r (FP22 mantissa, fp32 PSUM accumulation): 1 cyc/row
    at moving-dim >= 256, so projections aggregate at near peak;
  - softmax runs unnormalized (energies bounded ~|e|<50 so exp fits fp32);
    row-sums come free as two appended ones-columns in the aggregation
    matmul; normalization is folded into the PSUM->SBUF evacuation scale;
  - the column-pass diagonal mask is a single accumulating -1e30*I matmul
    on the tensor engine (no cross-engine hop);
  - chunk tails (aggregation/evac/DMA) are software-pipelined one chunk
    behind heads so the strict-FIFO engine queues never stall the PE.
"""

import numpy as np

import concourse.tile as tile
from concourse import bacc, mybir
from concourse.bass_utils import run_bass_kernel_spmd

B, C, H, W = 4, 512, 128, 128
CQK = C // 8          # 64
CV = C // 2           # 256 v channels per core
HW = H * W
N_CORES = 8

F32R = mybir.dt.float32r
F32 = mybir.dt.float32
F16 = mybir.dt.float16
EXP = mybir.ActivationFunctionType.Exp
COPY = mybir.ActivationFunctionType.Copy

_CACHE = {}

VBW = 258   # v buffer width: 256 channels + 2 ones columns


def _build(with_bias):
    nc = bacc.Bacc("TRN2", target_bir_lowering=False, debug=False,
                   num_devices=N_CORES)
    nck = 5 if with_bias else 4   # contraction chunks (last is the bias rows)
    xrows = C + (2 if with_bias else 0)

    xin = nc.dram_tensor("xin", [xrows, HW], F32R, kind="ExternalInput").ap()
    wqk = nc.dram_tensor("wqk", [xrows, 128], F32R, kind="ExternalInput").ap()
    wv = nc.dram_tensor("wv", [xrows, CV], F32R, kind="ExternalInput").ap()
    negid = nc.dram_tensor("negid", [128, 128], mybir.dt.bfloat16,
                           kind="ExternalInput").ap()
    id4 = nc.dram_tensor("id4", [128, 512], mybir.dt.bfloat16,
                         kind="ExternalInput").ap()
    ones2 = nc.dram_tensor("ones2", [128, 32], F32R, kind="ExternalInput").ap()

    vscr = nc.dram_tensor("vscr", [HW, CV], F32R).ap()
    orow = nc.dram_tensor("orow", [HW, CV], F16, kind="ExternalOutput").ap()
    ocol = nc.dram_tensor("ocol", [HW, CV], F16, kind="ExternalOutput").ap()
    rsr = nc.dram_tensor("rsr", [128, 128], F32, kind="ExternalOutput").ap()
    rsc = nc.dram_tensor("rsc", [128, 128], F32, kind="ExternalOutput").ap()

    with tile.TileContext(nc) as tc:
        with (
            tc.tile_pool(name="cst", bufs=1) as cst,
            tc.tile_pool(name="xs", bufs=2) as xsp,
            tc.tile_pool(name="p4", bufs=4) as p4p,
            tc.tile_pool(name="o16", bufs=2) as o16p,
            tc.tile_pool(name="psbig", bufs=2, space="PSUM") as psbigp,
            tc.tile_pool(name="psv", bufs=2, space="PSUM") as psvp,
            tc.tile_pool(name="psO", bufs=2, space="PSUM") as psOp,
        ):
            # prefetch the first two x chunks before anything else so the
            # first projection matmuls start as early as possible
            xpre = {}
            for ch0 in range(2):
                xs0 = []
                for k in range(nck):
                    rows = 128 if k < 4 else 2
                    xt = xsp.tile([128, 512], F32R, tag=f"xs{k}",
                                  name=f"xpre{ch0}_{k}")
                    nc.sync.dma_start(
                        xt[0:rows, :],
                        xin[k * 128:k * 128 + rows,
                            ch0 * 512:(ch0 + 1) * 512])
                    xs0.append(xt)
                xpre[ch0] = xs0

            # ---- persistent constants / accumulators ----
            WQK = cst.tile([128, nck * 128], F32R)
            for k in range(nck):
                rows = 128 if k < 4 else 2
                nc.sync.dma_start(WQK[0:rows, k * 128:k * 128 + 128],
                                  wqk[k * 128:k * 128 + rows, :])
            WV = cst.tile([128, nck * CV], F32R)
            for k in range(nck):
                rows = 128 if k < 4 else 2
                nc.sync.dma_start(WV[0:rows, k * CV:k * CV + CV],
                                  wv[k * 128:k * 128 + rows, :])
            NEGID = cst.tile([128, 128], mybir.dt.bfloat16)
            nc.sync.dma_start(NEGID[:], negid[:])
            ID4 = cst.tile([128, 512], mybir.dt.bfloat16)
            nc.sync.dma_start(ID4[:], id4[:])
            QK = cst.tile([128, HW], F32R)
            K2 = cst.tile([64, HW], F32R)
            RSR = cst.tile([128, 128], F32)
            RSC = cst.tile([128, 128], F32)

            # consolidated V buffers: 8 slots of [128, 258] per tensor,
            # ones columns (256:258 of each slot) loaded once
            VB = cst.tile([128, 8 * VBW], F32R)
            VTB = cst.tile([128, 16 * VBW], F32R)
            nc.sync.dma_start(
                VB[:].rearrange("p (s w) -> p s w", w=VBW)[:, :, 256:258],
                ones2[:, 0:16].rearrange("p (s w) -> p s w", w=2))
            nc.sync.dma_start(
                VTB[:].rearrange("p (s w) -> p s w", w=VBW)[:, :, 256:258],
                ones2[:].rearrange("p (s w) -> p s w", w=2))

            qk_of = QK[0:64, :].rearrange("c (y x) -> c y x", x=128)
            k2_of = K2[:].rearrange("c (y x) -> c y x", x=128)
            vscr_row4 = vscr.rearrange("(g t x) c -> g x t c", t=4, x=128)
            vscr_col4 = vscr.rearrange("(j g t) c -> g j t c", t=4, j=128)
            orow_4 = orow.rearrange("(g t x) c -> g x t c", t=4, x=128)
            ocol_4 = ocol.rearrange("(g t y) c -> g y t c", t=4, y=128)
            vb_slots = VB[:].rearrange("p (s w) -> p s w", w=VBW)
            vtb_slots = VTB[:].rearrange("p (s w) -> p s w", w=VBW)

            # =================== phase A + row pass ===================
            # software pipeline: chunk ch's aggregation tail is emitted
            # after chunk ch+1's head so exp/evac queueing never stalls PE
            row_state = {}

            def row_head(ch):
                csl = slice(ch * 512, (ch + 1) * 512)
                xsub = slice(0, 512)
                vbase = (ch % 2) * 4 * VBW
                xs = row_state.pop(("xs", ch))
                # qk projection for these 512 pixels
                pqk = psbigp.tile([128, 512], F32, tag="psbig")
                for k in range(nck):
                    rows = 128 if k < 4 else 2
                    nc.tensor.matmul(pqk[:],
                                     WQK[0:rows, k * 128:(k + 1) * 128],
                                     xs[k][0:rows, xsub],
                                     start=(k == 0), stop=(k == nck - 1))
                nc.scalar.activation(QK[:, csl], pqk[:], COPY)
                nc.vector.tensor_copy(K2[:, csl], QK[64:128, csl])

                pE = psbigp.tile([128, 512], F32, tag="psbig")
                for yy in range(4):
                    xsl = slice(yy * 128, yy * 128 + 128)
                    # v projection for row y -> [i, c]
                    pv = psvp.tile([128, CV], F32)
                    for k in range(nck):
                        rows = 128 if k < 4 else 2
                        nc.tensor.matmul(pv[:], xs[k][0:rows, xsl],
                                         WV[0:rows, k * CV:(k + 1) * CV],
                                         start=(k == 0),
                                         stop=(k == nck - 1))
                    nc.vector.tensor_copy(
                        VB[:, vbase + yy * VBW:vbase + yy * VBW + 256],
                        pv[:])
                for yy in range(4):
                    y = ch * 4 + yy
                    ysl = slice(y * 128, (y + 1) * 128)
                    # row energies E[i, x]
                    nc.tensor.matmul(pE[:, yy * 128:(yy + 1) * 128],
                                     K2[:, ysl], QK[0:64, ysl],
                                     start=True, stop=True)
                # batched v write: 4 rows at once
                nc.sync.dma_start(
                    vscr_row4[ch],
                    vb_slots[:, 4 * (ch % 2):4 * (ch % 2) + 4, 0:256])
                p4 = p4p.tile([128, 512], F32R)
                nc.scalar.activation(p4[:], pE[:], EXP)
                row_state[ch] = p4

            def row_tail(ch):
                vbase = (ch % 2) * 4 * VBW
                p4 = row_state.pop(ch)
                o16 = o16p.tile([128, 1024], F16, tag="o16r")
                for half in range(2):
                    pO = psOp.tile([128, 1024], F32)
                    for q2 in range(2):
                        yy = half * 2 + q2
                        nc.tensor.matmul(
                            pO[:, q2 * 512:q2 * 512 + VBW],
                            p4[:, yy * 128:(yy + 1) * 128],
                            VB[:, vbase + yy * VBW:vbase + (yy + 1) * VBW],
                            start=True, stop=True)
                    y0 = ch * 4 + half * 2
                    nc.vector.reciprocal(
                        RSR[:, y0:y0 + 2],
                        pO[:].rearrange("p (b k) -> p b k", k=512)[:, :, 256])
                    for q2 in range(2):
                        y = y0 + q2
                        nc.scalar.activation(
                            o16[:, (half * 2 + q2) * 256:
                                (half * 2 + q2 + 1) * 256],
                            pO[:, q2 * 512:q2 * 512 + 256], COPY,
                            scale=RSR[:, y:y + 1])
                nc.sync.dma_start(
                    orow_4[ch],
                    o16[:].rearrange("p (t c) -> p t c", c=256))

            def load_x(ch):
                xs = []
                for k in range(nck):
                    rows = 128 if k < 4 else 2
                    xt = xsp.tile([128, 512], F32R, tag=f"xs{k}")
                    nc.sync.dma_start(
                        xt[0:rows, :],
                        xin[k * 128:k * 128 + rows,
                            ch * 512:(ch + 1) * 512])
                    xs.append(xt)
                row_state[("xs", ch)] = xs

            row_state[("xs", 0)] = xpre[0]
            row_state[("xs", 1)] = xpre[1]
            for ch in range(33):
                if ch < 32:
                    if ch + 2 < 32:
                        load_x(ch + 2)
                    row_head(ch)
                if ch >= 1:
                    row_tail(ch - 1)

            # =================== column pass ===================
            # super-chunks of 8 columns: long back-to-back PE bursts keep
            # the HAM clock-gate open (see trn2 tensor-engine docs)
            vscr_col8 = vscr.rearrange("(j g t) c -> g j t c", t=8, j=128)
            ocol_8 = ocol.rearrange("(g t y) c -> g y t c", t=8, y=128)
            col_state = {}

            def col_head(sch):
                vbase = (sch % 2) * 8 * VBW
                # batched v column gather: 8 columns in one DMA
                nc.sync.dma_start(
                    vtb_slots[:, 8 * (sch % 2):8 * (sch % 2) + 8, 0:256],
                    vscr_col8[sch])
                p4s = []
                for g in range(2):
                    pE = psbigp.tile([128, 512], F32, tag="psbig")
                    for xx in range(4):
                        x = sch * 8 + g * 4 + xx
                        nc.tensor.matmul(pE[:, xx * 128:(xx + 1) * 128],
                                         k2_of[:, :, x], qk_of[:, :, x],
                                         start=(xx == 0), stop=False)
                    # mask the j==y diagonal of all 4 tiles
                    nc.tensor.matmul(pE[:], NEGID[:], ID4[:],
                                     start=False, stop=True)
                    p4 = p4p.tile([128, 512], F32R, tag="p4c")
                    nc.scalar.activation(p4[:], pE[:], EXP)
                    p4s.append(p4)
                col_state[sch] = p4s

            def col_tail(sch):
                vbase = (sch % 2) * 8 * VBW
                p4s = col_state.pop(sch)
                o16 = o16p.tile([128, 2048], F16, tag="o16c")
                for g in range(2):
                    p4 = p4s[g]
                    for half in range(2):
                        pO = psOp.tile([128, 1024], F32)
                        for q2 in range(2):
                            xx = half * 2 + q2
                            slot = g * 4 + xx
                            nc.tensor.matmul(
                                pO[:, q2 * 512:q2 * 512 + VBW],
                                p4[:, xx * 128:(xx + 1) * 128],
                                VTB[:, vbase + slot * VBW:
                                    vbase + (slot + 1) * VBW],
                                start=True, stop=True)
                        x0 = sch * 8 + g * 4 + half * 2
                        nc.vector.reciprocal(
                            RSC[:, x0:x0 + 2],
                            pO[:].rearrange("p (b k) -> p b k",
                                            k=512)[:, :, 256])
                        for q2 in range(2):
                            x = x0 + q2
                            oco = (g * 4 + half * 2 + q2) * 256
                            if q2 == 0:
                                nc.vector.tensor_scalar_mul(
                                    o16[:, oco:oco + 256],
                                    pO[:, q2 * 512:q2 * 512 + 256],
                                    RSC[:, x:x + 1])
                            else:
                                nc.scalar.activation(
                                    o16[:, oco:oco + 256],
                                    pO[:, q2 * 512:q2 * 512 + 256], COPY,
                                    scale=RSC[:, x:x + 1])
                nc.sync.dma_start(
                    ocol_8[sch],
                    o16[:].rearrange("p (t c) -> p t c", c=256))

            for sch in range(17):
                if sch < 16:
                    col_head(sch)
                if sch >= 1:
                    col_tail(sch - 1)

            nc.sync.dma_start(rsr[:], RSR[:])
            nc.sync.dma_start(rsc[:], RSC[:])

    nc.compile()
    return nc


def _get_nc(with_bias):
    key = bool(with_bias)
    if key not in _CACHE:
        _CACHE[key] = _build(key)
    return _CACHE[key]


def kernel(x, Wq, bq, Wk, bk, Wv, bv, _trace=False, _raw=False):
    x = np.asarray(x, np.float32)
    Wq = np.asarray(Wq, np.float32)
    Wk = np.asarray(Wk, np.float32)
    Wv = np.asarray(Wv, np.float32)
    bq = np.asarray(bq, np.float32)
    bk = np.asarray(bk, np.float32)
    bv = np.asarray(bv, np.float32)

    with_bias = bool(np.any(bq) or np.any(bk) or np.any(bv))
    nc = _get_nc(with_bias)

    import ml_dtypes
    negid_a = np.ascontiguousarray(
        (-1e30 * np.eye(128)).astype(ml_dtypes.bfloat16))
    id4_a = np.ascontiguousarray(
        np.tile(np.eye(128), (1, 4)).astype(ml_dtypes.bfloat16))
    ones2 = np.ones((128, 32), np.float32)
    wqk_full = np.concatenate([Wq.T, Wk.T], axis=1)       # [C, 128]
    if with_bias:
        bias_qk = np.concatenate([bq, bk])[None, :]       # [1, 128]
        wqk_full = np.concatenate(
            [wqk_full, bias_qk, np.zeros_like(bias_qk)], axis=0)

    in_maps = []
    for core in range(N_CORES):
        b, h = core // 2, core % 2
        xb = np.ascontiguousarray(x[b].reshape(C, HW))
        wvh = np.ascontiguousarray(Wv[h * CV:(h + 1) * CV, :].T)  # [C, CV]
        if with_bias:
            xb = np.concatenate([xb, np.ones((1, HW), np.float32),
                                 np.zeros((1, HW), np.float32)], axis=0)
            bvh = bv[h * CV:(h + 1) * CV][None, :]
            wvh = np.concatenate([wvh, bvh, np.zeros_like(bvh)], axis=0)
        in_maps.append({
            "xin": xb, "wqk": wqk_full, "wv": wvh,
            "negid": negid_a, "id4": id4_a, "ones2": ones2,
        })

    res = run_bass_kernel_spmd(nc, in_maps, list(range(N_CORES)),
                               trace=bool(_trace))
    if _raw:
        return res

    out = np.empty((B, C, H, W), np.float32)
    for core in range(N_CORES):
        b, h = core // 2, core % 2
        r = res.results[core]
        o_r = r["orow"].astype(np.float32).reshape(H, W, CV)   # [y, x, c]
        o_c = r["ocol"].astype(np.float32).reshape(W, H, CV)   # [x, y, c]
        a = r["rsr"].T        # [y, x] = 1/s_row
        bb = r["rsc"]         # [y, x] = 1/s_col
        w_r = (bb / (a + bb))[:, :, None]
        w_c = (a / (a + bb))[:, :, None]
        comb = o_r * w_r + o_c.transpose(1, 0, 2) * w_c        # [y, x, c]
        out[b, h * CV:(h + 1) * CV] = comb.transpose(2, 0, 1)

    if _trace:
        return out, res
    return out

